# revision 33
# baseline (speedup 1.0000x reference)
"""Multi-head attention (b=8, n=1024, dim=1024, heads=16) on 8 Trainium2 cores.

v4: row-group-interleaved S matmuls. The S (Q.K^T) matmuls contract over
only 64 partitions (head dim), so each occupies half the PE array rows
(tile_position (0,0) for the even head, (64,0) for the odd head, auto-
derived from base partitions). v3 emitted all 8 even-half steps then all 8
odd-half steps, so consecutive S matmuls hit the same row groups and each
paid ~100ns of exposed LDWEIGHTS (318ns vs 216ns) with zero overlap. v4
emits the two halves' S matmuls alternately each step: LDWEIGHTS for one
half hides under the other half's stream, and the two 64-deep streams run
concurrently in disjoint row groups.

Per-core structure (core b computes head-attention for batch row b):
  warmup:   ~44 dummy 128-col matmuls on a memset tile fill the PE from the
            end of its preamble (~7us) until the first input DMAs land
            (~11us), so HAM reaches K=8/8 before real work starts.
  V proj:   as v3 (wave-major over 4+3+1 psum accumulators, pair-0 q/k
            projections interleaved).
  attention: 8 pairs x 8 steps. Step (p,s) emits: S_e/S_o interleaved
            (4 matmuls, alternating row groups), AV rounds per a static
            schedule (s0: prev even head jt 6,7; s1-4: prev odd head
            jt 2(s-1),2(s-1)+1; s5-7: cur even head jt 2(s-5),2(s-5)+1;
            pair 7 also drains its odd head 1 round/step into the freed
            'proj' psum slot), q/k projection k-steps for pair p+1
            (2 k-steps/step), then the two exps (scalar engine only).
            One 'ot' psum accumulator serves all chains; the AV schedule
            never consumes a same-step pt.
  norm:     rowsums via the ones-column of v_aug (row 64 of ot psum),
            packed to rs_pack by DMA scatter; norm(p-1) emitted at (p,5);
            norm(7) hides under out-proj ch0 levels 0-6.
  out proj: ch0 hp-outer (levels 0-6 before norm(7)), ch1 it-outer.
"""

import numpy as np

N = 1024
D = 1024
H = 16
DH = 64
P = 128
SCALE = float(D) ** (-0.5)
NCORES = 8

_STATE: dict = {}


def _emit(tc, xT, wqkv, wout, bout, out):
    import concourse.mybir as mybir

    nc = tc.nc
    f32 = mybir.dt.float32
    bf16 = mybir.dt.bfloat16
    EXP = mybir.ActivationFunctionType.Exp

    from contextlib import ExitStack

    with ExitStack() as ctx:
        persist = ctx.enter_context(tc.tile_pool(name="persist", bufs=1))
        v_aug = [persist.tile([P, H * 65], bf16, tag=f"vaug{nt}", name=f"vaug{nt}")
                 for nt in range(8)]
        oT = [persist.tile([P, N], bf16, tag=f"oT{hp}", name=f"oT{hp}") for hp in range(8)]
        bias_sb = persist.tile([P, N], f32, tag="bias", name="bias_sb")
        bias2_sb = persist.tile([P, N], f32, tag="bias2", name="bias2_sb")
        # per-pair packed rowsums (16 partitions: head-even rows 0-7, odd 8-15)
        rs_pack = [persist.tile([H, P], f32, tag=f"rsp{i}", name=f"rs_pack{i}")
                   for i in range(2)]
        rcp_pack = [persist.tile([H, P], f32, tag=f"rcpp{i}", name=f"rcp_pack{i}")
                    for i in range(2)]
        ones_f = persist.tile([P, H], f32, tag="ones", name="ones_f")
        warm_sb = persist.tile([P, P], bf16, tag="warm", name="warm_sb")
        # staging for finished heads: one [65,N] read clears the ot psum WAR
        # in a single DVE op; oT copy + rowsum DMA then source from SBUF
        stg = [persist.tile([65, N], f32, tag=f"stg{i}", name=f"stg{i}")
               for i in range(2)]

        nc.gpsimd.memset(warm_sb[:], 0.0)
        nc.vector.memset(ones_f[:], 1.0)

        with tc.tile_pool(name="xt", bufs=1) as xt_pool, \
             tc.tile_pool(name="wv", bufs=1) as wv_pool, \
             tc.tile_pool(name="wstream", bufs=2) as wpool, \
             tc.tile_pool(name="wo", bufs=2) as wo_pool, \
             tc.tile_pool(name="qk", bufs=2) as qk_pool, \
             tc.tile_pool(name="p", bufs=1) as p_pool, \
             tc.tile_pool(name="bc", bufs=2) as bc_pool, \
             tc.tile_pool(name="osb", bufs=2) as o_pool, \
             tc.tile_pool(name="dbounce", bufs=1, space="DRAM") as d_pool, \
             tc.tile_pool(name="ps", bufs=1, space="PSUM") as sp:

            dram_t2 = d_pool.tile([H, N], f32, tag="d2", name="dram_t2")

            # ---------------- PE warmup ----------------
            # dummy 128-col matmuls on the zeroed tile: no DMA deps, so they
            # run from the end of the PE preamble until real inputs land,
            # pushing HAM to K=8/8 before the first V-proj matmul
            warm_ps = sp.tile([P, N], f32, tag="proj", name="warm_ps")
            for i in range(44):
                nc.tensor.matmul(warm_ps[:, 0:P], lhsT=warm_sb[:], rhs=warm_sb[:],
                                 start=True, stop=True)

            # ---------------- input DMAs ----------------
            # x/wv round-robin on sync+gpsimd so arrival order tracks k; wq/wk
            # get their own queues (scalar/vector) so the pair-0 q/k projection
            # k-steps interleave with V instead of waiting for the x/wv stream
            q2 = [nc.sync, nc.gpsimd]
            xT_sb, wv_sb = [], []
            for k in range(8):
                w = wv_pool.tile([P, N], bf16, tag=f"wv{k}", name=f"wv{k}")
                q2[k % 2].dma_start(w[:], wqkv[k * P:(k + 1) * P, 2048:3072])
                wv_sb.append(w)
                t = xt_pool.tile([P, N], bf16, tag=f"xt{k}", name=f"xt{k}")
                q2[(k + 1) % 2].dma_start(t[:], xT[k * P:(k + 1) * P, :])
                xT_sb.append(t)

            def load_w(lo, eng):
                tiles = []
                for k in range(8):
                    w = wpool.tile([P, 512], bf16, tag=f"w{k}", name=f"w{k}")
                    eng.dma_start(w[:], wqkv[k * P:(k + 1) * P, lo:lo + 512])
                    tiles.append(w)
                return tiles

            wq_c = load_w(0, nc.scalar)
            wk_c = load_w(1024, nc.scalar)
            nc.scalar.dma_start(bias_sb[:], bout[0:1, :].broadcast_to([P, N]))
            # doubled ch0 bias: [bias(0:512) | bias(0:512)] for the paired
            # [128,1024] finish adds (bias after wq/wk: only needed at tail)
            for bh in range(2):
                nc.scalar.dma_start(bias2_sb[:, bh * 512:(bh + 1) * 512],
                                    bout[0:1, 0:512].broadcast_to([P, 512]))

            # ones column of v_aug
            for nt in range(8):
                nc.vector.tensor_copy(
                    v_aug[nt][:].rearrange("p (h e) -> p h e", e=65)[:, :, 64:65],
                    ones_f[:, :, None])

            def alloc_qk():
                q = qk_pool.tile([P, N], bf16, tag="q", name="qTt")
                k = qk_pool.tile([P, N], bf16, tag="k", name="kTt")
                return q, k

            def proj_mm(ps_t, wt, off, k):
                # one k-step of a projection chunk: both 512-col halves
                for ic in range(2):
                    nc.tensor.matmul(
                        ps_t[:, ic * 512:(ic + 1) * 512],
                        lhsT=wt[k][:, off:off + P],
                        rhs=xT_sb[k][:, ic * 512:(ic + 1) * 512],
                        start=(k == 0), stop=(k == 7))

            def v_mm(nt, ps_t, k):
                for ic in range(2):
                    nc.tensor.matmul(
                        ps_t[:, ic * 512:(ic + 1) * 512],
                        lhsT=xT_sb[k][:, nt * P:(nt + 1) * P],
                        rhs=wv_sb[k][:, ic * 512:(ic + 1) * 512],
                        start=(k == 0), stop=(k == 7))

            def v_copy(nt, ps_t):
                nc.vector.tensor_copy(
                    v_aug[nt][:].rearrange("p (h e) -> p h e", e=65)[:, :, 0:64],
                    ps_t[:].rearrange("p (h e) -> p h e", e=64))

            # ---------------- V projection + pair-0 prologue ----------------
            # group 1: V chunks 0-3 wave-major over 4 psum accumulators (as
            # (xT[k], wv[k]) arrive, all in-flight chunks advance one k-step);
            # group 2: V chunk 4 with the pair-0 q projection; group 3: the
            # pair-0 k projection (gated by the wk DMA tail). V chunks 5-7
            # move into pair-0's steps — attention start is DMA-bound, not
            # PE-bound, so their 10us of matmuls would otherwise serialize.
            qk_cur = alloc_qk()
            vps = [sp.tile([P, N], f32, tag=t, name="v_ps")
                   for t in ("sps0", "sps1", "ot", "proj")]
            for k in range(8):
                for c in range(4):
                    v_mm(c, vps[c], k)
            for c in range(4):
                v_copy(c, vps[c])

            v4_ps = sp.tile([P, N], f32, tag="ot", name="v4_ps")
            q_ps = sp.tile([P, N], f32, tag="proj", name="q_ps")
            for k in range(8):
                v_mm(4, v4_ps, k)
                proj_mm(q_ps, wq_c, 0, k)
            v_copy(4, v4_ps)
            nc.scalar.copy(qk_cur[0][:], q_ps[:])

            k_ps = sp.tile([P, N], f32, tag="sps1", name="k_ps")
            for k in range(8):
                proj_mm(k_ps, wk_c, 0, k)
            nc.scalar.copy(qk_cur[1][:], k_ps[:])

            # ---------------- attention ----------------
            # pt tiles are tagged statically by (half, step): even tiles are
            # consumed within the same pair (reuse one pair later is safe);
            # odd tiles for steps 0-1 are consumed only at (p+1, 1), after
            # the same-step production of pair p+1, so those two tags are
            # doubled by pair parity.
            def pt_tag(p, hf, s):
                if hf == 0:
                    return f"pe{s}"
                if s <= 1:
                    return f"po{p % 2}{s}"
                return f"po{s}"

            pend = {}          # (h, jt) -> pt tile
            acc = {}           # h -> psum accumulator tile
            staged = set()

            def emit_av(h, jt, slot_tag, c1_eng=None):
                if jt == 0:
                    acc[h] = sp.tile([P, N], f32, tag=slot_tag, name=f"ot_{h}")
                pt = pend.pop((h, jt))
                ot_t = acc[h]
                va = v_aug[jt][:].rearrange("p (h e) -> p h e", e=65)[:, h, :]
                for ic in range(2):
                    nc.tensor.matmul(ot_t[0:65, ic * 512:(ic + 1) * 512], lhsT=va,
                                     rhs=pt[:, ic * 512:(ic + 1) * 512],
                                     start=(jt == 0), stop=(jt == 7))
                if jt == 7:
                    emit_stage(h, c1_eng=c1_eng)

            def emit_stage(h, c1_eng=None):
                # single [65,N] psum read (frees the ot slot after one DVE
                # op), then oT rows and the rowsum-row DMA source from SBUF
                ot_t = acc.pop(h)
                pr, hf = divmod(h, 2)
                st = stg[h % 2]
                if c1_eng is nc.scalar:
                    nc.scalar.copy(st[:], ot_t[0:65, :])
                else:
                    nc.vector.tensor_copy(st[:], ot_t[0:65, :])
                # oT fan-out on gpsimd (SBUF->SBUF): keeps the DVE queue clear
                # for reciprocals / norm multiplies / tail bias-adds
                nc.gpsimd.tensor_copy(oT[pr][64 * hf:64 * hf + 64, :], st[0:64, :])
                nc.gpsimd.dma_start(rs_pack[pr % 2][8 * hf:8 * hf + 8, :],
                                    st[64:65, :])
                staged.add(h)
                emit_norm_half(h)

            bc_cur = {}

            def emit_norm_half(h):
                # per-head half of the normalization chain, emitted right
                # after stage(h) so the reciprocal -> DRAM bounce -> broadcast
                # DMA latency (~5us serial) spreads across the following steps
                pr, hf = divmod(h, 2)
                rp = rs_pack[pr % 2]
                cp = rcp_pack[pr % 2]
                # full-16-row reciprocal: DVE ops must start at partition 0;
                # the other half's rows are stale-but-harmless, and only this
                # head's 8 rows are broadcast below
                nc.vector.reciprocal(cp[0:H, :], rp[0:H, :])
                if hf == 0:
                    bc_cur[pr] = bc_pool.tile([P, N], f32, tag=f"bc{pr % 2}", name="bc")
                # DRAM bounce for the partition broadcast (SBUF sources cannot
                # have a zero-stride partition dim)
                eng = nc.sync if hf == 0 else nc.gpsimd
                eng.dma_start(
                    dram_t2[h:h + 1, :].rearrange("a (b c) -> (a b) c", b=8),
                    cp[8 * hf:8 * hf + 8, :])
                eng.dma_start(
                    bc_cur[pr][64 * hf:64 * hf + 64, :],
                    dram_t2[h:h + 1, :].broadcast_to([64, N]))

            def emit_norm_mul(p):
                nc.vector.tensor_mul(oT[p][:], oT[p][:], bc_cur.pop(p)[:])

            def load_wo(ch, eng):
                tiles = []
                for hp in range(8):
                    w = wo_pool.tile([P, 512], bf16, tag=f"wo{hp}", name=f"wo{hp}")
                    eng.dma_start(w[:], wout[hp * P:(hp + 1) * P, ch * 512:(ch + 1) * 512])
                    tiles.append(w)
                return tiles

            # AV rounds drained at step (p, s), in consumption order. Pair 0's
            # even-head rounds start at s6 (its 'ot' slot hosts the in-loop V
            # chunks until then), spilling jt 5-7 into pair 1's step 0.
            def av_rounds(p, s):
                rds = []
                if p == 1 and s == 0:
                    return [(0, 5), (0, 6), (0, 7)]
                if p >= 1:
                    if s == 0:
                        rds += [(2 * p - 2, 7)]
                    elif s <= 4:
                        rds += [(2 * p - 1, 2 * (s - 1)), (2 * p - 1, 2 * s - 1)]
                if p == 0:
                    if s == 6:
                        rds += [(0, 0), (0, 1)]
                    elif s == 7:
                        rds += [(0, 2), (0, 3), (0, 4)]
                    return rds
                if s >= 5:
                    rds += [(2 * p, 2 * (s - 5)), (2 * p, 2 * s - 9)]
                if s == 7:
                    rds += [(2 * p, 6)]
                return rds

            for p in range(8):
                qT_c, kT_c = qk_cur
                if p == 3:
                    wq_c1 = load_w(512, nc.sync)
                    wk_c1 = load_w(1024 + 512, nc.gpsimd)
                if p == 6:
                    # prefetch output-projection weights so they are resident
                    # before the sync queue fills with tail bc broadcasts
                    wo0 = load_wo(0, nc.sync)
                    wo1 = load_wo(1, nc.gpsimd)
                if p < 7:
                    qk_next = alloc_qk()
                    wq_n = wq_c if (p + 1) < 4 else wq_c1
                    wk_n = wk_c if (p + 1) < 4 else wk_c1
                    off_n = ((p + 1) % 4) * P
                for s in range(8):
                    def emit_s():
                        # S for both halves, row-group interleaved: even half
                        # in rows 0-63 (tile (0,0)), odd in 64-127 ((64,0));
                        # alternation overlaps the streams and hides LDWEIGHTS
                        sps_e = sp.tile([P, N], f32, tag="sps0", name="sps_e")
                        sps_o = sp.tile([P, N], f32, tag="sps1", name="sps_o")
                        for ic in range(2):
                            nc.tensor.matmul(
                                sps_e[:, ic * 512:(ic + 1) * 512],
                                lhsT=kT_c[0:64, s * P:(s + 1) * P],
                                rhs=qT_c[0:64, ic * 512:(ic + 1) * 512],
                                start=True, stop=True)
                            nc.tensor.matmul(
                                sps_o[:, ic * 512:(ic + 1) * 512],
                                lhsT=kT_c[64:128, s * P:(s + 1) * P],
                                rhs=qT_c[64:128, ic * 512:(ic + 1) * 512],
                                start=True, stop=True)
                        return sps_e, sps_o

                    # pair-0 steps 0-5 host V chunks 5-7 in the idle 'ot' slot
                    # (4 k-steps per step, copy to v_aug after the second)
                    if p == 0 and s < 6:
                        ch = 5 + s // 2
                        if s % 2 == 0:
                            vch_ps = sp.tile([P, N], f32, tag="ot", name="vch_ps")
                        for k in range(4 * (s % 2), 4 * (s % 2) + 4):
                            v_mm(ch, vch_ps, k)
                        if s % 2 == 1:
                            v_copy(ch, vch_ps)
                    sps_e, sps_o = emit_s()
                    for (h, jt) in av_rounds(p, s):
                        emit_av(h, jt, "ot")
                    if p == 7 and s >= 2:
                        emit_av(15, s - 2, "proj")
                    # projection k-steps for pair p+1: q on steps 0-3
                    # (2/step), k on 4-6 (3/3/2) so the k cast lands a full
                    # step before the next pair's S needs it
                    if p < 7:
                        KS = [(0, (0, 1)), (0, (2, 3)), (0, (4, 5)), (0, (6, 7)),
                              (1, (0, 1, 2)), (1, (3, 4, 5)), (1, (6, 7)), None]
                        if KS[s] is not None:
                            c, kks = KS[s]
                            if kks[0] == 0:
                                proj_ps = sp.tile([P, N], f32, tag="proj",
                                                  name="proj_ps")
                            wt = wq_n if c == 0 else wk_n
                            for kk in kks:
                                proj_mm(proj_ps, wt, off_n, kk)
                            if kks[-1] == 7:
                                # cast in 512-col halves: finer-grained deps
                                nc.vector.tensor_copy(qk_next[c][:, 0:512],
                                                      proj_ps[:, 0:512])
                                nc.vector.tensor_copy(qk_next[c][:, 512:1024],
                                                      proj_ps[:, 512:1024])
                    if p >= 1 and s == 6:
                        emit_norm_mul(p - 1)
                    # exps last: the scalar engine runs just behind the PE
                    pt_e = p_pool.tile([P, N], bf16, tag=pt_tag(p, 0, s), name="pt_e")
                    nc.scalar.activation(pt_e[:], sps_e[:], EXP, scale=SCALE)
                    pend[(2 * p, s)] = pt_e
                    pt_o = p_pool.tile([P, N], bf16, tag=pt_tag(p, 1, s), name="pt_o")
                    nc.scalar.activation(pt_o[:], sps_o[:], EXP, scale=SCALE)
                    pend[(2 * p + 1, s)] = pt_o
                if p < 7:
                    qk_cur = qk_next

            # epilogue: finish head 14 (ot slot) and head 15 (proj slot);
            # their [65,N] stage reads run on the now-idle scalar engine
            emit_av(14, 7, "ot", c1_eng=nc.scalar)
            emit_av(15, 6, "proj")
            emit_av(15, 7, "proj", c1_eng=nc.scalar)

            # ---------------- output projection ----------------
            def finish_tile(it, ch, ps_t, eng):
                osb = o_pool.tile([P, 512], f32, tag=f"o{it % 2}", name="o_sb")
                nc.vector.tensor_add(osb[:], ps_t,
                                     bias_sb[0:P, ch * 512:(ch + 1) * 512])
                eng.dma_start(out[it * P:(it + 1) * P, ch * 512:(ch + 1) * 512], osb[:])

            def finish_pair(it2, full_ps, eng0, eng1):
                # one [128,1024] bias-add covering two ch0 it-tiles (halves
                # the serial DVE chain gating the ch1 entry), two row DMAs
                osb = o_pool.tile([P, N], f32, tag=f"op{it2 % 2}", name="o_pair")
                nc.vector.tensor_add(osb[:], full_ps, bias2_sb[:])
                eng0.dma_start(out[2 * it2 * P:(2 * it2 + 1) * P, 0:512],
                               osb[:, 0:512])
                eng1.dma_start(out[(2 * it2 + 1) * P:(2 * it2 + 2) * P, 0:512],
                               osb[:, 512:1024])

            # ch0 hp-outer. Levels 0-6 are emitted BEFORE the pair-7
            # normalization so they cannot inherit its semaphore wait. Slot
            # order = availability order: sps0/sps1 free after the last exps,
            # ot after stage(14), proj after stage(15); those stages complete
            # during the first levels' matmuls.
            fslots, ftiles = [], []
            for tg in ("sps0", "sps1", "ot", "proj"):
                t = sp.tile([P, N], f32, tag=tg, name="f_ps")
                ftiles.append(t)
                fslots.append(t[:, 0:512])
                fslots.append(t[:, 512:1024])
            for hp in range(7):
                for it in range(8):
                    nc.tensor.matmul(
                        fslots[it], lhsT=oT[hp][:, it * P:(it + 1) * P],
                        rhs=wo0[hp][:], start=(hp == 0), stop=False)

            # LDWEIGHTS-only bridge: keeps the PE activity monitor fed while
            # the hp7 level waits out the tail of the pair-7 norm chain
            for i in range(24):
                nc.tensor.ldweights(warm_sb[:, 0:P])

            # pair-7 normalization, split by head: head 14's chain completes
            # ~2us before head 15's, so its half-level matmuls (64-deep,
            # row group 0-1) run while head 15's broadcast drains; head 15's
            # halves then overlap via the (64,0) row tile
            bc7 = bc_cur.pop(7)
            nc.vector.tensor_mul(oT[7][0:64, :], oT[7][0:64, :], bc7[0:64, :])
            nc.vector.tensor_mul(oT[7][64:128, :], oT[7][64:128, :],
                                 bc7[64:128, :])
            for it in range(8):
                nc.tensor.matmul(
                    fslots[it], lhsT=oT[7][0:64, it * P:(it + 1) * P],
                    rhs=wo0[7][0:64, :], start=False, stop=False)
            for it in range(8):
                nc.tensor.matmul(
                    fslots[it], lhsT=oT[7][64:128, it * P:(it + 1) * P],
                    rhs=wo0[7][64:128, :], start=False, stop=True)
            for it2, ftile in enumerate(ftiles):
                finish_pair(it2, ftile[:], nc.sync, nc.gpsimd)

            # ch1: it-outer, rotating over freed slots in finish order
            for it in range(8):
                ps_t = sp.tile([P, 512], f32, tag=["sps0", "sps1", "ot"][it % 3],
                               name="f1_ps")
                for hp in range(8):
                    nc.tensor.matmul(
                        ps_t[:], lhsT=oT[hp][:, it * P:(it + 1) * P],
                        rhs=wo1[hp][:], start=(hp == 0), stop=(hp == 7))
                finish_tile(it, 1, ps_t[:], nc.sync)


def build(mm_dtype: str = "bfloat16"):
    key = ("nc", mm_dtype)
    if key in _STATE:
        return _STATE[key]
    import concourse.mybir as mybir
    import concourse.tile as tile
    from concourse import bacc

    nc = bacc.Bacc("TRN2", target_bir_lowering=False, debug=False,
                   enable_asserts=False, num_devices=NCORES)
    f32 = mybir.dt.float32
    bf16 = mybir.dt.bfloat16
    xT = nc.dram_tensor("xT", [D, N], bf16, kind="ExternalInput").ap()
    wqkv = nc.dram_tensor("wqkv", [D, 3 * D], bf16, kind="ExternalInput").ap()
    wout = nc.dram_tensor("wout", [D, D], bf16, kind="ExternalInput").ap()
    bout = nc.dram_tensor("bout", [1, D], f32, kind="ExternalInput").ap()
    out = nc.dram_tensor("out", [N, D], f32, kind="ExternalOutput").ap()

    with tile.TileContext(nc) as tc:
        _emit(tc, xT, wqkv, wout, bout, out)
    nc.compile()
    _STATE[key] = nc
    return nc


def make_in_maps(x, w_qkv, w_out, b_out):
    import ml_dtypes
    bf = ml_dtypes.bfloat16
    x = np.asarray(x, np.float32)
    w_qkv = np.ascontiguousarray(np.asarray(w_qkv, np.float32)).astype(bf)
    w_out = np.ascontiguousarray(np.asarray(w_out, np.float32)).astype(bf)
    b_out = np.ascontiguousarray(np.asarray(b_out, np.float32)).reshape(1, D)
    return [
        {"xT": np.ascontiguousarray(x[b].T).astype(bf), "wqkv": w_qkv,
         "wout": w_out, "bout": b_out}
        for b in range(x.shape[0])
    ]


def run(x, w_qkv, w_out, b_out, trace=False, mm_dtype="bfloat16"):
    from concourse.bass_utils import run_bass_kernel_spmd

    nc = build(mm_dtype)
    in_maps = make_in_maps(x, w_qkv, w_out, b_out)
    res = run_bass_kernel_spmd(nc, in_maps, core_ids=list(range(NCORES)), trace=trace)
    outs = np.stack([res.results[c]["out"] for c in range(NCORES)])
    return outs, res


def kernel(x, w_qkv, w_out, b_out):
    outs, _ = run(x, w_qkv, w_out, b_out, trace=False)
    return outs.astype(np.float32)
